# revision 1
# baseline (speedup 1.0000x reference)
"""Bass/Tile TRN2 kernel v2 for nn_CrossAttentionLayer (B=8, NQ=64, S=4096, D=1024, H=16).

Sharding: pure data-parallel - core b computes batch element b. No collectives.

Reassociated formulation (saves ~60k PE rows vs materializing K and V):
  K-side:  scores_h = Q~_h Wk_h C^T  with M_h = Q~_h Wk_h computed first (tiny).
           scoresT [s, hn] = C-stationary x M^T-moving GEMM. bk drops out
           entirely (softmax shift invariance).
  V-side:  out_h = (P_h C) Wv_h^T + bv_h.  U^T [d, hn] is produced directly
           (C-natural stationary x PT moving), accumulated in SBUF f32.
           Rowsums: sum_f32 += PT (DVE) per s-subtile, then two ap-512
           ones-column matmuls reduce over partitions; reciprocal is broadcast
           back to [128, hn] via a 1-partition matmul so normalization is a
           plain elementwise DVE mul on U^T.  Per-head Wv projection is
           software-pipelined one t behind the U^T accumulation in the last
           block; final wo projection identical to baseline.
"""

import numpy as np
import ml_dtypes
from contextlib import ExitStack

import concourse.bass as bass
import concourse.tile as tile
from concourse import bacc, mybir
from concourse.bass_utils import run_bass_kernel_spmd

B, NQ, S, D = 8, 64, 4096, 1024
H, HD = 16, 64
N_CORES = 8
SCALE = float(HD) ** -0.5

BF16 = mybir.dt.bfloat16
F32 = mybir.dt.float32
NPBF16 = ml_dtypes.bfloat16
AF = mybir.ActivationFunctionType

S_BLK = 1024
N_BLK = S // S_BLK
N_SUB = S_BLK // 128   # 128-row s-subtiles per block
DT = D // 128          # 128-wide d tiles
NPAIR = H // 2

_PROGRAM = None


def _emit(ctx: ExitStack, tc: tile.TileContext, aps: dict, dbg: dict | None = None, repeat: int = 1):
    nc = tc.nc
    qT = aps["qT"]
    wqT, wkn = aps["wqT"], aps["wkn"]
    bqr = aps["bqr"]

    const = ctx.enter_context(tc.tile_pool(name="const", bufs=1))
    wpool = ctx.enter_context(tc.tile_pool(name="wpool", bufs=1))
    # scratch ring: wqT+wkn tiles early, utn tiles late (same shape/dtype)
    scr = ctx.enter_context(tc.tile_pool(name="scr", bufs=16))
    ctxp = ctx.enter_context(tc.tile_pool(name="ctxp", bufs=12))
    cnp = ctx.enter_context(tc.tile_pool(name="cnp", bufs=12))
    ptp = ctx.enter_context(tc.tile_pool(name="ptp", bufs=10))
    work = ctx.enter_context(tc.tile_pool(name="work", bufs=1))
    psum_sc = ctx.enter_context(tc.tile_pool(name="psum_sc", bufs=4, space="PSUM"))
    # one-bank tiles only: [128, 1024] f32 (2-bank) psum tiles fail walrus codegen
    psum_u = ctx.enter_context(tc.tile_pool(name="psum_u", bufs=4, space="PSUM"))

    # ---- prologue input loads (weights for the epilogue load at block 1) ----
    qt_in = [const.tile([128, NQ], BF16, tag=f"qtin{t}", name=f"qtin{t}") for t in range(DT)]
    for t in range(DT):
        nc.sync.dma_start(qt_in[t][:], qT[128 * t : 128 * (t + 1), :])
    wq_sb = [scr.tile([128, D], BF16, tag="scr", name=f"wq{t}") for t in range(DT)]
    for t in range(DT):
        nc.sync.dma_start(wq_sb[t][:], wqT[128 * t : 128 * (t + 1), :])
    wk_sb = [scr.tile([128, D], BF16, tag="scr", name=f"wk{t}") for t in range(DT)]
    for t in range(DT):
        nc.sync.dma_start(wk_sb[t][:], wkn[128 * t : 128 * (t + 1), :])
    bq_sb = const.tile([128, DT], F32, tag="bq", name="bq_sb")
    nc.sync.dma_start(bq_sb[:], bqr[:, :])

    wv_sb = [wpool.tile([128, D], BF16, tag=f"wv{t}", name=f"wv{t}") for t in range(DT)]
    wo_sb = [wpool.tile([128, D], BF16, tag=f"wo{t}", name=f"wo{t}") for t in range(DT)]
    bv_sb = const.tile([128, DT], F32, tag="bv", name="bv_sb")
    bo_bc = const.tile([NQ, D], F32, tag="bo_bc", name="bo_bc")

    ones_col = const.tile([128, 1], BF16, tag="ones_col", name="ones_col")
    nc.vector.memset(ones_col[:], 1.0)
    ones_bf = const.tile([1, 128], BF16, tag="ones_bf", name="ones_bf")
    nc.vector.memset(ones_bf[:], 1.0)

    # ---- Q~^T = wq x^T + bq, packed block-diagonally per head pair ----
    # qt2[t] = [[Q~_{2t}^T, 0], [0, Q~_{2t+1}^T]] (128 x 128, bf16).  Mixing
    # tile_position row-bases inside one psum accumulation group crashes the
    # NEFF at runtime, so M^T uses full-128 contraction with zero-padding to
    # kill cross-head terms (the baseline-proven pattern).
    qt2 = [const.tile([128, 128], BF16, tag=f"qt{t}", name=f"qt{t}") for t in range(DT)]
    for t in range(DT):
        nc.vector.memset(qt2[t][:], 0.0)
        ps = psum_sc.tile([128, NQ], F32, tag="sc", name="sc_ps")
        for d in range(DT):
            nc.tensor.matmul(
                ps[:],
                wq_sb[d][:, 128 * t : 128 * (t + 1)],
                qt_in[d][:],
                start=(d == 0),
                stop=(d == DT - 1),
            )
        nc.scalar.activation(
            qt2[t][0:64, 0:64], ps[0:64, :], AF.Identity, bias=bq_sb[0:64, t : t + 1]
        )
        nc.scalar.activation(
            qt2[t][64:128, 64:128], ps[64:128, :], AF.Identity,
            bias=bq_sb[64:128, t : t + 1],
        )

    # ---- M^T[d, 64h+n] = sum_j Wk_h[j, d] Q~^T[64h+j, n] : mt[dc] [128 d, 1024 hn]
    # pair-block t contributes mt[dc][:, 128t:128t+128] in one full-contraction
    # matmul: wk_sb[t] rows are (head 2t | head 2t+1) dims, qt2's zero blocks
    # remove the cross-head products.
    mt = [const.tile([128, D], BF16, tag=f"mt{dc}", name=f"mt{dc}") for dc in range(DT)]
    for dc in range(DT):
        for g in range(2):
            ps = psum_sc.tile([128, 512], F32, tag="sc", name="sc_ps")
            for k in range(4):
                t = 4 * g + k
                nc.tensor.matmul(
                    ps[:, 128 * k : 128 * (k + 1)],
                    wk_sb[t][:, 128 * dc : 128 * (dc + 1)],
                    qt2[t][:],
                    start=(k == 0),
                    stop=(k == 3),
                )
            nc.scalar.activation(
                mt[dc][:, 512 * g : 512 * (g + 1)], ps[:], AF.Identity
            )

    # ---- U^T accumulators + rowsum accumulator + reciprocal broadcast ----
    accT = [const.tile([128, D], F32, tag=f"acc{t}", name=f"acc{t}") for t in range(DT)]
    sum_f32 = const.tile([128, D], F32, tag="sumf", name="sum_f32")
    rbc = const.tile([128, D], BF16, tag="rbc", name="rbc")

    env = dict(locals())
    env["w_loaded"] = [False, False]
    for _rep in range(repeat):
        _emit_body(tc, aps, dbg if _rep == 0 else None, env)


def _emit_body(tc: tile.TileContext, aps: dict, dbg: dict | None, env: dict):
    nc = tc.nc
    ctxT, cnat, out = aps["ctxT"], aps["cnat"], aps["out"]
    ctxp, cnp, ptp, scr, work = env["ctxp"], env["cnp"], env["ptp"], env["scr"], env["work"]
    psum_sc, psum_u = env["psum_sc"], env["psum_u"]
    mt, accT, sum_f32, rbc = env["mt"], env["accT"], env["sum_f32"], env["rbc"]
    wv_sb, wo_sb = env["wv_sb"], env["wo_sb"]
    bv_sb, bo_bc = env["bv_sb"], env["bo_bc"]
    ones_col, ones_bf = env["ones_col"], env["ones_bf"]
    const = env["const"]

    nc.vector.memset(sum_f32[:], 0.0)

    utn = [None] * DT
    ps_wv_box = [None, None]

    def emit_wv(t):
        for hp in range(NPAIR):
            for sub in range(2):
                h = 2 * hp + sub
                ps = ps_wv_box[sub]
                nc.tensor.matmul(
                    ps[64 * sub : 64 * (sub + 1), 64 * hp : 64 * (hp + 1)],
                    wv_sb[t][:, 64 * h : 64 * (h + 1)],
                    utn[t][:, 64 * h : 64 * (h + 1)],
                    start=(t == 0 and hp == 0),
                    stop=(t == DT - 1 and hp == NPAIR - 1),
                )

    for blk in range(N_BLK):
        s0 = blk * S_BLK
        ctx_sb = [ctxp.tile([128, S_BLK], BF16, tag="ctx", name="ctx_t") for _ in range(DT)]
        for d in range(DT):
            nc.sync.dma_start(
                ctx_sb[d][:], ctxT[128 * d : 128 * (d + 1), s0 : s0 + S_BLK]
            )
        cn_sb = [cnp.tile([128, D], BF16, tag="cn", name="cn_t") for _ in range(N_SUB)]
        for si in range(N_SUB):
            r0 = s0 + 128 * si
            nc.sync.dma_start(cn_sb[si][:], cnat[r0 : r0 + 128, :])

        # epilogue weights stream in behind the block prefetches: wv at block 1,
        # wo at block 2 — spreading the 4MB halves the peak DMA-queue backlog
        if blk == 1 and not env["w_loaded"][0]:
            env["w_loaded"][0] = True
            for t in range(DT):
                nc.sync.dma_start(wv_sb[t][:], aps["wvT"][128 * t : 128 * (t + 1), :])
            nc.sync.dma_start(bv_sb[:], aps["bvr"][:, :])
            nc.sync.dma_start(bo_bc[:], aps["bobc"][:, :])
        if blk == 2 and not env["w_loaded"][1]:
            env["w_loaded"][1] = True
            for t in range(DT):
                nc.sync.dma_start(wo_sb[t][:], aps["woT"][128 * t : 128 * (t + 1), :])

        # scoresT + exp: pt[si] [128 s, 1024 hn] bf16
        pt = [ptp.tile([128, D], BF16, tag="pt", name="pt_t") for _ in range(N_SUB)]
        for si in range(N_SUB):
            for half in range(2):
                ps = psum_sc.tile([128, 512], F32, tag="sc", name="sc_ps")
                for d in range(DT):
                    nc.tensor.matmul(
                        ps[:],
                        ctx_sb[d][:, 128 * si : 128 * (si + 1)],
                        mt[d][:, 512 * half : 512 * (half + 1)],
                        start=(d == 0),
                        stop=(d == DT - 1),
                    )
                nc.scalar.activation(
                    pt[si][:, 512 * half : 512 * (half + 1)], ps[:], AF.Exp, scale=SCALE
                )
            nc.vector.tensor_add(sum_f32[:], sum_f32[:], pt[si][:])

        if dbg is not None and blk == 0:
            nc.sync.dma_start(dbg["pt0"][:, :], pt[0][:])

        last = blk == N_BLK - 1

        # U^T[t] += sum_si C-natural[si]-slice^T @ PT[si]
        for t in range(DT):
            psu = [
                psum_u.tile([128, 512], F32, tag="u", name="u_ps") for _ in range(2)
            ]
            for si in range(N_SUB):
                for c in range(2):
                    nc.tensor.matmul(
                        psu[c][:],
                        cn_sb[si][:, 128 * t : 128 * (t + 1)],
                        pt[si][:, 512 * c : 512 * (c + 1)],
                        start=(si == 0),
                        stop=(si == N_SUB - 1),
                    )
            for c in range(2):
                acc_sl = accT[t][:, 512 * c : 512 * (c + 1)]
                if blk == 0:
                    nc.vector.tensor_copy(acc_sl, psu[c][:])
                else:
                    nc.vector.tensor_add(acc_sl, psu[c][:], acc_sl)

            if last and t == 0:
                # rowsums -> reciprocal row -> broadcast [128, 1024] (PE is busy
                # with U^T(0) while the DVE finishes sum_bf, so no PE stall)
                sum_bf = work.tile([128, D], BF16, tag="sumbf", name="sum_bf")
                nc.vector.tensor_copy(sum_bf[:], sum_f32[:])
                rcT = work.tile([1, D], BF16, tag="rcT", name="rcT")
                for c in range(2):
                    psr = psum_sc.tile([1, 512], F32, tag="sc", name="sc_ps")
                    nc.tensor.matmul(
                        psr[:], ones_col[:], sum_bf[:, 512 * c : 512 * (c + 1)],
                        start=True, stop=True,
                    )
                    # bf16 reciprocal: ~0.2% on 1/r, i.e. ~0.2% relative on the
                    # output's own magnitude - far inside the 2e-2 budget
                    with nc.allow_low_precision(reason="bf16 recip of rowsums"):
                        nc.vector.reciprocal(rcT[:, 512 * c : 512 * (c + 1)], psr[:])
                for c in range(2):
                    psb = psum_sc.tile([128, 512], F32, tag="sc", name="sc_ps")
                    nc.tensor.matmul(
                        psb[:], ones_bf[:], rcT[:, 512 * c : 512 * (c + 1)],
                        start=True, stop=True,
                    )
                    nc.vector.tensor_copy(rbc[:, 512 * c : 512 * (c + 1)], psb[:])
                # allocated AFTER the rowsum/broadcast tiles so the 4-slot
                # ring can't hand these held accumulators' banks to them;
                # two tiles (one per head-of-pair) so the partition-split
                # accumulation chains live in separate psum banks/groups
                ps_wv_box[0] = psum_sc.tile([64, 512], F32, tag="sc", name="sc_ps")
                ps_wv_box[1] = psum_sc.tile([128, 512], F32, tag="sc", name="sc_ps")

            if last:
                utn[t] = scr.tile([128, D], BF16, tag="scr", name=f"utn{t}")
                nc.vector.tensor_mul(utn[t][:], accT[t][:], rbc[:])

            if last and t >= 1:
                emit_wv(t - 1)
        if last:
            emit_wv(DT - 1)

    if dbg is not None:
        nc.sync.dma_start(dbg["mt0"][:, :], mt[0][:])
        nc.sync.dma_start(dbg["acc0"][:, :], accT[0][:])
        nc.sync.dma_start(dbg["sumf"][:, :], sum_f32[:])
        nc.sync.dma_start(dbg["rbc"][:, :], rbc[:])
        nc.sync.dma_start(dbg["ut0"][:, :], utn[0][:])

    # ---- evict o^T pairs with bv bias ----
    ots = []
    for hp in range(NPAIR):
        ot = const.tile([128, NQ], BF16, tag=f"ot{hp}", name=f"ot{hp}")
        nc.scalar.activation(
            ot[0:64, :], ps_wv_box[0][0:64, 64 * hp : 64 * (hp + 1)],
            AF.Identity, bias=bv_sb[0:64, hp : hp + 1],
        )
        nc.scalar.activation(
            ot[64:128, :], ps_wv_box[1][64:128, 64 * hp : 64 * (hp + 1)],
            AF.Identity, bias=bv_sb[64:128, hp : hp + 1],
        )
        ots.append(ot)
        if dbg is not None and hp == 0:
            nc.sync.dma_start(dbg["ot0"][:, :], ot[:])

    # ---- output projection ----
    out_sb = const.tile([NQ, D], F32, tag="out_sb", name="out_sb")
    for c in range(2):
        ps = psum_sc.tile([NQ, 512], F32, tag="sc", name="sc_ps")
        for t in range(DT):
            nc.tensor.matmul(
                ps[:],
                ots[t][:],
                wo_sb[t][:, 512 * c : 512 * (c + 1)],
                start=(t == 0),
                stop=(t == DT - 1),
            )
        nc.vector.tensor_add(
            out_sb[:, 512 * c : 512 * (c + 1)], ps[:], bo_bc[:, 512 * c : 512 * (c + 1)]
        )
    nc.sync.dma_start(out[:, :], out_sb[:])


DBG_SHAPES = {
    "mt0": ([128, D], BF16),
    "pt0": ([128, D], BF16),
    "acc0": ([128, D], F32),
    "sumf": ([128, D], F32),
    "rbc": ([128, D], BF16),
    "ut0": ([128, D], BF16),
    "ot0": ([128, NQ], BF16),
}


def _build_program(debug_dumps: bool = False, repeat: int = 1):
    nc = bacc.Bacc("TRN2", target_bir_lowering=False, debug=False)
    aps = {
        "ctxT": nc.dram_tensor("ctxT", [D, S], BF16, kind="ExternalInput").ap(),
        "cnat": nc.dram_tensor("cnat", [S, D], BF16, kind="ExternalInput").ap(),
        "qT": nc.dram_tensor("qT", [D, NQ], BF16, kind="ExternalInput").ap(),
        "wqT": nc.dram_tensor("wqT", [D, D], BF16, kind="ExternalInput").ap(),
        "wkn": nc.dram_tensor("wkn", [D, D], BF16, kind="ExternalInput").ap(),
        "wvT": nc.dram_tensor("wvT", [D, D], BF16, kind="ExternalInput").ap(),
        "woT": nc.dram_tensor("woT", [D, D], BF16, kind="ExternalInput").ap(),
        "bqr": nc.dram_tensor("bqr", [128, DT], F32, kind="ExternalInput").ap(),
        "bvr": nc.dram_tensor("bvr", [128, DT], F32, kind="ExternalInput").ap(),
        "bobc": nc.dram_tensor("bobc", [NQ, D], F32, kind="ExternalInput").ap(),
        "out": nc.dram_tensor("out", [NQ, D], F32, kind="ExternalOutput").ap(),
    }
    dbg = None
    if debug_dumps:
        dbg = {
            k: nc.dram_tensor(f"dbg_{k}", shp, dt, kind="ExternalOutput").ap()
            for k, (shp, dt) in DBG_SHAPES.items()
        }
    with tile.TileContext(nc) as tc:
        with ExitStack() as stack:
            _emit(stack, tc, aps, dbg, repeat=repeat)
    nc.compile()
    return nc


def _get_program():
    global _PROGRAM
    if _PROGRAM is None:
        _PROGRAM = _build_program()
    return _PROGRAM


def make_in_maps(inputs: dict) -> list[dict]:
    q = np.asarray(inputs["queries"], np.float32)
    ctxf = np.asarray(inputs["context"], np.float32)
    shared = {}
    for wname, key, transpose in (
        ("wq", "wqT", True), ("wk", "wkn", False), ("wv", "wvT", True), ("wo", "woT", True),
    ):
        w = np.asarray(inputs[wname], np.float32)
        shared[key] = (w.T if transpose else w).astype(NPBF16, order="C")
    shared["bqr"] = np.ascontiguousarray(
        np.asarray(inputs["bq"], np.float32).reshape(DT, 128).T
    )
    shared["bvr"] = np.ascontiguousarray(
        np.asarray(inputs["bv"], np.float32).reshape(DT, 128).T
    )
    shared["bobc"] = np.ascontiguousarray(
        np.broadcast_to(np.asarray(inputs["bo"], np.float32).reshape(1, D), (NQ, D))
    )

    in_maps = []
    for b in range(B):
        m = dict(shared)
        cb = ctxf[b].astype(NPBF16, order="C")
        m["cnat"] = cb
        m["ctxT"] = np.ascontiguousarray(cb.T)
        m["qT"] = q[b].T.astype(NPBF16, order="C")
        in_maps.append(m)
    return in_maps


def kernel(**inputs) -> np.ndarray:
    nc = _get_program()
    in_maps = make_in_maps(inputs)
    res = run_bass_kernel_spmd(nc, in_maps, core_ids=list(range(N_CORES)))
    return np.stack([res.results[b]["out"] for b in range(B)]).astype(np.float32)



# revision 7
# speedup vs baseline: 1.9140x; 1.9140x over previous
"""Bass/Tile TRN2 kernel v2 for nn_CrossAttentionLayer (B=8, NQ=64, S=4096, D=1024, H=16).

Sharding: pure data-parallel - core b computes batch element b. No collectives.

Reassociated formulation (saves ~60k PE rows vs materializing K and V):
  K-side:  scores_h = Q~_h Wk_h C^T  with M_h = Q~_h Wk_h computed first (tiny).
           scoresT [s, hn] = C-stationary x M^T-moving GEMM. bk drops out
           entirely (softmax shift invariance).
  V-side:  out_h = (P_h C) Wv_h^T + bv_h.  U^T [d, hn] is produced directly
           (C-natural stationary x PT moving), accumulated in SBUF f32.
           Rowsums: sum_f32 += PT (DVE) per s-subtile, then two ap-512
           ones-column matmuls reduce over partitions; reciprocal is broadcast
           back to [128, hn] via a 1-partition matmul so normalization is a
           plain elementwise DVE mul on U^T.  Per-head Wv projection is
           software-pipelined one t behind the U^T accumulation in the last
           block; final wo projection identical to baseline.
"""

import numpy as np
import ml_dtypes
from contextlib import ExitStack

import concourse.bass as bass
import concourse.tile as tile
from concourse import bacc, mybir
from concourse.bass_utils import run_bass_kernel_spmd

B, NQ, S, D = 8, 64, 4096, 1024
H, HD = 16, 64
N_CORES = 8
SCALE = float(HD) ** -0.5

BF16 = mybir.dt.bfloat16
F32 = mybir.dt.float32
NPBF16 = ml_dtypes.bfloat16
AF = mybir.ActivationFunctionType

S_BLK = 1024
N_BLK = S // S_BLK
N_SUB = S_BLK // 128   # 128-row s-subtiles per block
DT = D // 128          # 128-wide d tiles
NPAIR = H // 2

_PROGRAM = None


def _emit(ctx: ExitStack, tc: tile.TileContext, aps: dict, dbg: dict | None = None, repeat: int = 1):
    nc = tc.nc
    qT = aps["qT"]
    wqT, wkn = aps["wqT"], aps["wkn"]
    bqr = aps["bqr"]

    const = ctx.enter_context(tc.tile_pool(name="const", bufs=1))
    wpool = ctx.enter_context(tc.tile_pool(name="wpool", bufs=1))
    # scratch ring: wq then wk staged sequentially early, utn tiles late
    scr = ctx.enter_context(tc.tile_pool(name="scr", bufs=8))
    ctxp = ctx.enter_context(tc.tile_pool(name="ctxp", bufs=16))
    cnp = ctx.enter_context(tc.tile_pool(name="cnp", bufs=16))
    ptp = ctx.enter_context(tc.tile_pool(name="ptp", bufs=10))
    work = ctx.enter_context(tc.tile_pool(name="work", bufs=1))
    # scores ring: ACT evicts (~600ns) well inside the 8-MM group (~1700ns),
    # so 2 banks suffice and the ring is never held across a body boundary
    psum_sc = ctx.enter_context(tc.tile_pool(name="psum_sc", bufs=2, space="PSUM"))
    # one-bank tiles only: [128, 1024] f32 (2-bank) psum tiles fail walrus codegen
    psum_u = ctx.enter_context(tc.tile_pool(name="psum_u", bufs=4, space="PSUM"))
    # dedicated banks for the wv accumulators + out projection so the scores
    # ring is free the moment the next body starts
    psum_wv = ctx.enter_context(tc.tile_pool(name="psum_wv", bufs=2, space="PSUM"))

    # ---- prologue input loads (weights for the epilogue load at block 1) ----
    qt_in = [const.tile([128, NQ], BF16, tag=f"qtin{t}", name=f"qtin{t}") for t in range(DT)]
    for t in range(DT):
        nc.sync.dma_start(qt_in[t][:], qT[128 * t : 128 * (t + 1), :])
    wq_sb = [scr.tile([128, D], BF16, tag="scr", name=f"wq{t}") for t in range(DT)]
    for t in range(DT):
        nc.sync.dma_start(wq_sb[t][:], wqT[128 * t : 128 * (t + 1), :])
    bq_sb = const.tile([128, DT], F32, tag="bq", name="bq_sb")
    nc.sync.dma_start(bq_sb[:], bqr[:, :])

    wv_sb = [wpool.tile([128, D], BF16, tag=f"wv{t}", name=f"wv{t}") for t in range(DT)]
    wo_sb = [wpool.tile([128, D], BF16, tag=f"wo{t}", name=f"wo{t}") for t in range(DT)]
    bv_sb = const.tile([128, DT], F32, tag="bv", name="bv_sb")
    bo_bc = const.tile([NQ, D], F32, tag="bo_bc", name="bo_bc")

    ones_col = const.tile([128, 1], BF16, tag="ones_col", name="ones_col")
    nc.vector.memset(ones_col[:], 1.0)
    ones_bf = const.tile([1, 128], BF16, tag="ones_bf", name="ones_bf")
    nc.vector.memset(ones_bf[:], 1.0)

    # ---- Q~^T = wq x^T + bq, packed block-diagonally per head pair ----
    # qt2[t] = [[Q~_{2t}^T, 0], [0, Q~_{2t+1}^T]] (128 x 128, bf16).  Mixing
    # tile_position row-bases inside one psum accumulation group crashes the
    # NEFF at runtime, so M^T uses full-128 contraction with zero-padding to
    # kill cross-head terms (the baseline-proven pattern).
    qt2 = [const.tile([128, 128], BF16, tag=f"qt{t}", name=f"qt{t}") for t in range(DT)]
    for t in range(DT):
        nc.vector.memset(qt2[t][:], 0.0)
        ps = psum_sc.tile([128, NQ], F32, tag="sc", name="sc_ps")
        for d in range(DT):
            nc.tensor.matmul(
                ps[:],
                wq_sb[d][:, 128 * t : 128 * (t + 1)],
                qt_in[d][:],
                start=(d == 0),
                stop=(d == DT - 1),
            )
        nc.scalar.activation(
            qt2[t][0:64, 0:64], ps[0:64, :], AF.Identity, bias=bq_sb[0:64, t : t + 1]
        )
        nc.scalar.activation(
            qt2[t][64:128, 64:128], ps[64:128, :], AF.Identity,
            bias=bq_sb[64:128, t : t + 1],
        )

    # wk staged after qt2 consumed the wq tiles: the 8-buf scr ring recycles
    wk_sb = [scr.tile([128, D], BF16, tag="scr", name=f"wk{t}") for t in range(DT)]
    for t in range(DT):
        nc.sync.dma_start(wk_sb[t][:], wkn[128 * t : 128 * (t + 1), :])

    # ---- M^T[d, 64h+n] = sum_j Wk_h[j, d] Q~^T[64h+j, n] : mt[dc] [128 d, 1024 hn]
    # pair-block t contributes mt[dc][:, 128t:128t+128] in one full-contraction
    # matmul: wk_sb[t] rows are (head 2t | head 2t+1) dims, qt2's zero blocks
    # remove the cross-head products.
    mt = [const.tile([128, D], BF16, tag=f"mt{dc}", name=f"mt{dc}") for dc in range(DT)]
    for dc in range(DT):
        for g in range(2):
            ps = psum_sc.tile([128, 512], F32, tag="sc", name="sc_ps")
            for k in range(4):
                t = 4 * g + k
                nc.tensor.matmul(
                    ps[:, 128 * k : 128 * (k + 1)],
                    wk_sb[t][:, 128 * dc : 128 * (dc + 1)],
                    qt2[t][:],
                    start=(k == 0),
                    stop=(k == 3),
                )
            nc.scalar.activation(
                mt[dc][:, 512 * g : 512 * (g + 1)], ps[:], AF.Identity
            )

    # ---- U^T accumulators + rowsum accumulator + reciprocal broadcast ----
    accT = [const.tile([128, D], F32, tag=f"acc{t}", name=f"acc{t}") for t in range(DT)]
    sum_f32 = const.tile([128, D], F32, tag="sumf", name="sum_f32")
    rbc = const.tile([128, D], BF16, tag="rbc", name="rbc")

    env = dict(locals())
    env["w_loaded"] = [False, False]
    for _rep in range(repeat):
        _emit_body(tc, aps, dbg if _rep == 0 else None, env)


def _emit_body(tc: tile.TileContext, aps: dict, dbg: dict | None, env: dict):
    nc = tc.nc
    ctxT, cnat, out = aps["ctxT"], aps["cnat"], aps["out"]
    ctxp, cnp, ptp, scr, work = env["ctxp"], env["cnp"], env["ptp"], env["scr"], env["work"]
    psum_sc, psum_u, psum_wv = env["psum_sc"], env["psum_u"], env["psum_wv"]
    mt, accT, sum_f32, rbc = env["mt"], env["accT"], env["sum_f32"], env["rbc"]
    wv_sb, wo_sb = env["wv_sb"], env["wo_sb"]
    bv_sb, bo_bc = env["bv_sb"], env["bo_bc"]
    ones_col, ones_bf = env["ones_col"], env["ones_bf"]
    const = env["const"]

    nc.vector.memset(sum_f32[:], 0.0)

    utn = [None] * DT
    ps_wv_box = [None, None]

    def emit_wv(t):
        for hp in range(NPAIR):
            for sub in range(2):
                h = 2 * hp + sub
                ps = ps_wv_box[sub]
                nc.tensor.matmul(
                    ps[64 * sub : 64 * (sub + 1), 64 * hp : 64 * (hp + 1)],
                    wv_sb[t][:, 64 * h : 64 * (h + 1)],
                    utn[t][:, 64 * h : 64 * (h + 1)],
                    start=(t == 0 and hp == 0),
                    stop=(t == DT - 1 and hp == NPAIR - 1),
                )

    for blk in range(N_BLK):
        s0 = blk * S_BLK
        ctx_sb = [ctxp.tile([128, S_BLK], BF16, tag="ctx", name="ctx_t") for _ in range(DT)]
        for d in range(DT):
            nc.sync.dma_start(
                ctx_sb[d][:], ctxT[128 * d : 128 * (d + 1), s0 : s0 + S_BLK]
            )
        cn_sb = [cnp.tile([128, D], BF16, tag="cn", name="cn_t") for _ in range(N_SUB)]
        for si in range(N_SUB):
            r0 = s0 + 128 * si
            nc.sync.dma_start(cn_sb[si][:], cnat[r0 : r0 + 128, :])

        # epilogue weights stream in behind the block prefetches: wv at block 1,
        # wo at block 2 — spreading the 4MB halves the peak DMA-queue backlog
        if blk == 1 and not env["w_loaded"][0]:
            env["w_loaded"][0] = True
            for t in range(DT):
                nc.sync.dma_start(wv_sb[t][:], aps["wvT"][128 * t : 128 * (t + 1), :])
            nc.sync.dma_start(bv_sb[:], aps["bvr"][:, :])
            nc.sync.dma_start(bo_bc[:], aps["bobc"][:, :])
        if blk == 2 and not env["w_loaded"][1]:
            env["w_loaded"][1] = True
            for t in range(DT):
                nc.sync.dma_start(wo_sb[t][:], aps["woT"][128 * t : 128 * (t + 1), :])

        # scoresT + exp: pt[si] [128 s, 1024 hn] bf16
        pt = [ptp.tile([128, D], BF16, tag="pt", name="pt_t") for _ in range(N_SUB)]
        for si in range(N_SUB):
            for half in range(2):
                ps = psum_sc.tile([128, 512], F32, tag="sc", name="sc_ps")
                for d in range(DT):
                    nc.tensor.matmul(
                        ps[:],
                        ctx_sb[d][:, 128 * si : 128 * (si + 1)],
                        mt[d][:, 512 * half : 512 * (half + 1)],
                        start=(d == 0),
                        stop=(d == DT - 1),
                    )
                nc.scalar.activation(
                    pt[si][:, 512 * half : 512 * (half + 1)], ps[:], AF.Exp, scale=SCALE
                )
            nc.vector.tensor_add(sum_f32[:], sum_f32[:], pt[si][:])

        if dbg is not None and blk == 0:
            nc.sync.dma_start(dbg["pt0"][:, :], pt[0][:])

        last = blk == N_BLK - 1

        # U^T[t] += sum_si C-natural[si]-slice^T @ PT[si]
        for t in range(DT):
            psu = [
                psum_u.tile([128, 512], F32, tag="u", name="u_ps") for _ in range(2)
            ]
            for si in range(N_SUB):
                for c in range(2):
                    nc.tensor.matmul(
                        psu[c][:],
                        cn_sb[si][:, 128 * t : 128 * (t + 1)],
                        pt[si][:, 512 * c : 512 * (c + 1)],
                        start=(si == 0),
                        stop=(si == N_SUB - 1),
                    )
            for c in range(2):
                acc_sl = accT[t][:, 512 * c : 512 * (c + 1)]
                if blk == 0:
                    nc.vector.tensor_copy(acc_sl, psu[c][:])
                else:
                    nc.vector.tensor_add(acc_sl, psu[c][:], acc_sl)

            if last and t == 0:
                # rowsums -> reciprocal row -> broadcast [128, 1024] (PE is busy
                # with U^T(0) while the DVE finishes sum_bf, so no PE stall)
                sum_bf = work.tile([128, D], BF16, tag="sumbf", name="sum_bf")
                nc.vector.tensor_copy(sum_bf[:], sum_f32[:])
                rcT = work.tile([1, D], BF16, tag="rcT", name="rcT")
                for c in range(2):
                    psr = psum_sc.tile([1, 512], F32, tag="sc", name="sc_ps")
                    nc.tensor.matmul(
                        psr[:], ones_col[:], sum_bf[:, 512 * c : 512 * (c + 1)],
                        start=True, stop=True,
                    )
                    # bf16 reciprocal: ~0.2% on 1/r, i.e. ~0.2% relative on the
                    # output's own magnitude - far inside the 2e-2 budget
                    with nc.allow_low_precision(reason="bf16 recip of rowsums"):
                        nc.vector.reciprocal(rcT[:, 512 * c : 512 * (c + 1)], psr[:])
                for c in range(2):
                    psb = psum_sc.tile([128, 512], F32, tag="sc", name="sc_ps")
                    nc.tensor.matmul(
                        psb[:], ones_bf[:], rcT[:, 512 * c : 512 * (c + 1)],
                        start=True, stop=True,
                    )
                    nc.vector.tensor_copy(rbc[:, 512 * c : 512 * (c + 1)], psb[:])
                # dedicated banks: holding these in the scores ring would
                # stall the next body's first scores groups; two tiles (one
                # per head-of-pair) so the partition-split accumulation
                # chains live in separate psum banks/groups
                ps_wv_box[0] = psum_wv.tile([64, 512], F32, tag="wv", name="wv_ps")
                ps_wv_box[1] = psum_wv.tile([128, 512], F32, tag="wv", name="wv_ps")

            if last:
                utn[t] = scr.tile([128, D], BF16, tag="scr", name=f"utn{t}")
                nc.vector.tensor_mul(utn[t][:], accT[t][:], rbc[:])

            if last and t >= 1:
                emit_wv(t - 1)
        if last:
            emit_wv(DT - 1)

    if dbg is not None:
        nc.sync.dma_start(dbg["mt0"][:, :], mt[0][:])
        nc.sync.dma_start(dbg["acc0"][:, :], accT[0][:])
        nc.sync.dma_start(dbg["sumf"][:, :], sum_f32[:])
        nc.sync.dma_start(dbg["rbc"][:, :], rbc[:])
        nc.sync.dma_start(dbg["ut0"][:, :], utn[0][:])

    # ---- evict o^T pairs with bv bias ----
    ots = []
    for hp in range(NPAIR):
        ot = const.tile([128, NQ], BF16, tag=f"ot{hp}", name=f"ot{hp}")
        nc.scalar.activation(
            ot[0:64, :], ps_wv_box[0][0:64, 64 * hp : 64 * (hp + 1)],
            AF.Identity, bias=bv_sb[0:64, hp : hp + 1],
        )
        nc.scalar.activation(
            ot[64:128, :], ps_wv_box[1][64:128, 64 * hp : 64 * (hp + 1)],
            AF.Identity, bias=bv_sb[64:128, hp : hp + 1],
        )
        ots.append(ot)
        if dbg is not None and hp == 0:
            nc.sync.dma_start(dbg["ot0"][:, :], ot[:])

    # ---- output projection ----
    out_sb = const.tile([NQ, D], F32, tag="out_sb", name="out_sb")
    for c in range(2):
        ps = psum_wv.tile([NQ, 512], F32, tag="wv", name="wv_ps")
        for t in range(DT):
            nc.tensor.matmul(
                ps[:],
                ots[t][:],
                wo_sb[t][:, 512 * c : 512 * (c + 1)],
                start=(t == 0),
                stop=(t == DT - 1),
            )
        nc.vector.tensor_add(
            out_sb[:, 512 * c : 512 * (c + 1)], ps[:], bo_bc[:, 512 * c : 512 * (c + 1)]
        )
    nc.sync.dma_start(out[:, :], out_sb[:])


DBG_SHAPES = {
    "mt0": ([128, D], BF16),
    "pt0": ([128, D], BF16),
    "acc0": ([128, D], F32),
    "sumf": ([128, D], F32),
    "rbc": ([128, D], BF16),
    "ut0": ([128, D], BF16),
    "ot0": ([128, NQ], BF16),
}


def _build_program(debug_dumps: bool = False, repeat: int = 1):
    nc = bacc.Bacc("TRN2", target_bir_lowering=False, debug=False)
    aps = {
        "ctxT": nc.dram_tensor("ctxT", [D, S], BF16, kind="ExternalInput").ap(),
        "cnat": nc.dram_tensor("cnat", [S, D], BF16, kind="ExternalInput").ap(),
        "qT": nc.dram_tensor("qT", [D, NQ], BF16, kind="ExternalInput").ap(),
        "wqT": nc.dram_tensor("wqT", [D, D], BF16, kind="ExternalInput").ap(),
        "wkn": nc.dram_tensor("wkn", [D, D], BF16, kind="ExternalInput").ap(),
        "wvT": nc.dram_tensor("wvT", [D, D], BF16, kind="ExternalInput").ap(),
        "woT": nc.dram_tensor("woT", [D, D], BF16, kind="ExternalInput").ap(),
        "bqr": nc.dram_tensor("bqr", [128, DT], F32, kind="ExternalInput").ap(),
        "bvr": nc.dram_tensor("bvr", [128, DT], F32, kind="ExternalInput").ap(),
        "bobc": nc.dram_tensor("bobc", [NQ, D], F32, kind="ExternalInput").ap(),
        "out": nc.dram_tensor("out", [NQ, D], F32, kind="ExternalOutput").ap(),
    }
    dbg = None
    if debug_dumps:
        dbg = {
            k: nc.dram_tensor(f"dbg_{k}", shp, dt, kind="ExternalOutput").ap()
            for k, (shp, dt) in DBG_SHAPES.items()
        }
    with tile.TileContext(nc) as tc:
        with ExitStack() as stack:
            _emit(stack, tc, aps, dbg, repeat=repeat)
    nc.compile()
    return nc


def _get_program():
    global _PROGRAM
    if _PROGRAM is None:
        _PROGRAM = _build_program()
    return _PROGRAM


def make_in_maps(inputs: dict) -> list[dict]:
    q = np.asarray(inputs["queries"], np.float32)
    ctxf = np.asarray(inputs["context"], np.float32)
    shared = {}
    for wname, key, transpose in (
        ("wq", "wqT", True), ("wk", "wkn", False), ("wv", "wvT", True), ("wo", "woT", True),
    ):
        w = np.asarray(inputs[wname], np.float32)
        shared[key] = (w.T if transpose else w).astype(NPBF16, order="C")
    shared["bqr"] = np.ascontiguousarray(
        np.asarray(inputs["bq"], np.float32).reshape(DT, 128).T
    )
    shared["bvr"] = np.ascontiguousarray(
        np.asarray(inputs["bv"], np.float32).reshape(DT, 128).T
    )
    shared["bobc"] = np.ascontiguousarray(
        np.broadcast_to(np.asarray(inputs["bo"], np.float32).reshape(1, D), (NQ, D))
    )

    in_maps = []
    for b in range(B):
        m = dict(shared)
        cb = ctxf[b].astype(NPBF16, order="C")
        m["cnat"] = cb
        m["ctxT"] = np.ascontiguousarray(cb.T)
        m["qT"] = q[b].T.astype(NPBF16, order="C")
        in_maps.append(m)
    return in_maps


def kernel(**inputs) -> np.ndarray:
    nc = _get_program()
    in_maps = make_in_maps(inputs)
    res = run_bass_kernel_spmd(nc, in_maps, core_ids=list(range(N_CORES)))
    return np.stack([res.results[b]["out"] for b in range(B)]).astype(np.float32)

